# revision 1
# baseline (speedup 1.0000x reference)
"""Cross-attention kernel for TRN2, SPMD over 8 NeuronCores.

Problem: B=8, SQ=4096, SKV=77, D_EMBED=1024, D_CROSS=768, H=16, DH=64.
  q = x @ wq + bq ; k = y @ wk + bk ; v = y @ wv + bv
  out = softmax(q k^T / 8) v @ wo + bo

Sharding: pure data-parallel over batch (1 batch element per core, no
collectives). Host pre-transposes x and y per core so the device kernel
keeps every tensor feature-major (contraction dim on partitions) until the
O-projection, which uses attnout^T as the stationary operand to emit the
output in natural row-major layout.

Compute dtype: bf16 operands (host-cast), fp32 PSUM accumulation, fp32 out.

Device layout per core (all DRAM tensors are per-core inputs):
  xt  (1024, 4096) bf16 : x[b]^T
  yt  (768, 77)    bf16 : y[b]^T
  wq/wo (1024,1024), wk/wv (768,1024) bf16
  bq/bv/bo (1, 1024) bf16 ; bk8 (1, 1024) f32 = bk * 0.125
  out (4096, 1024) f32

Softmax is computed without max-subtraction (scores are O(5) for this
problem class; exp stays comfortably inside fp32/bf16 range):
  scoresT[s,q] = k'_h @ q_h^T with k' = (k + bk)/8 folded at k-projection
  e = exp(scoresT)  (bf16)
  r = 1 / (ones77 @ e)          per-head [1, SQ] via PE column-sum
  aoT[d,q] = (v_h^T @ e) * broadcast(r)   (normalization commutes)
  out[q,:] = aoT^T @ wo + bo    (aoT tiles as stationary operand)
"""

import numpy as np
import ml_dtypes

import concourse.bass as bass
import concourse.mybir as mybir
import concourse.tile as tile
from concourse import bacc
from concourse import bass_utils

F32 = mybir.dt.float32
BF16 = mybir.dt.bfloat16
AF = mybir.ActivationFunctionType

B = 8
SQ = 4096
SKV = 77
D = 1024
DC = 768
H = 16
DH = 64
KT = D // 128    # 8 embed k-tiles
KC = DC // 128   # 6 cross k-tiles
CT = D // 128    # 8 column tiles of the 1024-wide projections
CH = 512         # query chunk
NCH = SQ // CH   # 8 chunks
NQT = CH // 128  # 4 query 128-tiles per chunk

_CACHED = {}


def _build():
    nc = bacc.Bacc("TRN2", target_bir_lowering=False, debug=False, num_devices=B)

    xt = nc.dram_tensor("xt", (D, SQ), BF16, kind="ExternalInput")
    yt = nc.dram_tensor("yt", (DC, SKV), BF16, kind="ExternalInput")
    wq_d = nc.dram_tensor("wq", (D, D), BF16, kind="ExternalInput")
    wk_d = nc.dram_tensor("wk", (DC, D), BF16, kind="ExternalInput")
    wv_d = nc.dram_tensor("wv", (DC, D), BF16, kind="ExternalInput")
    wo_d = nc.dram_tensor("wo", (D, D), BF16, kind="ExternalInput")
    bq_d = nc.dram_tensor("bq", (1, D), BF16, kind="ExternalInput")
    bk8_d = nc.dram_tensor("bk8", (1, D), F32, kind="ExternalInput")
    bv_d = nc.dram_tensor("bv", (1, D), BF16, kind="ExternalInput")
    bo_d = nc.dram_tensor("bo", (1, D), BF16, kind="ExternalInput")
    sel16_d = nc.dram_tensor("sel16", (SKV, H * 16), BF16, kind="ExternalInput")
    sel64_d = nc.dram_tensor("sel64", (16, H * DH), BF16, kind="ExternalInput")
    out_d = nc.dram_tensor("out", (SQ, D), F32, kind="ExternalOutput")

    with tile.TileContext(nc) as tc:
        with (
            tc.tile_pool(name="consts", bufs=1) as consts,
            tc.tile_pool(name="wpool", bufs=1) as wpool,
            tc.tile_pool(name="xpool", bufs=2) as xpool,
            tc.tile_pool(name="qpool", bufs=2) as qpool,
            tc.tile_pool(name="epool", bufs=2) as epool,
            tc.tile_pool(name="rp", bufs=4) as rp,
            tc.tile_pool(name="rbpool", bufs=2) as rbpool,
            tc.tile_pool(name="aopool", bufs=2) as aopool,
            tc.tile_pool(name="opool", bufs=3) as opool,
            tc.tile_pool(name="pmm", bufs=2, space="PSUM") as pmm,
            tc.tile_pool(name="psc", bufs=2, space="PSUM") as psc,
            tc.tile_pool(name="ppv", bufs=2, space="PSUM") as ppv,
            tc.tile_pool(name="pnrm", bufs=2, space="PSUM") as pnrm,
        ):
            # ---- constants / weights ----
            wq_sb = wpool.tile([128, KT, D], BF16, tag="wq")
            nc.sync.dma_start(wq_sb[:], wq_d.ap().rearrange("(kt p) n -> p kt n", p=128))
            wo_sb = wpool.tile([128, KT, D], BF16, tag="wo")
            nc.sync.dma_start(wo_sb[:], wo_d.ap().rearrange("(kt p) n -> p kt n", p=128))
            wk_sb = wpool.tile([128, KC, D], BF16, tag="wk")
            nc.sync.dma_start(wk_sb[:], wk_d.ap().rearrange("(kt p) n -> p kt n", p=128))
            wv_sb = wpool.tile([128, KC, D], BF16, tag="wv")
            nc.sync.dma_start(wv_sb[:], wv_d.ap().rearrange("(kt p) n -> p kt n", p=128))

            yt_sb = consts.tile([128, KC, SKV], BF16, tag="yt")
            nc.sync.dma_start(yt_sb[:], yt.ap().rearrange("(kt p) s -> p kt s", p=128))

            bq_sb = consts.tile([1, D], BF16, tag="bq")
            nc.sync.dma_start(bq_sb[:], bq_d.ap())
            bv_sb = consts.tile([1, D], BF16, tag="bv")
            nc.sync.dma_start(bv_sb[:], bv_d.ap())
            bo_sb = consts.tile([1, D], BF16, tag="bo")
            nc.sync.dma_start(bo_sb[:], bo_d.ap())
            bk8_sb = consts.tile([128, CT], F32, tag="bk8")
            nc.sync.dma_start(bk8_sb[:], bk8_d.ap().rearrange("a (ct p) -> (a p) ct", p=128))

            ones_q = consts.tile([1, CH], BF16, tag="ones_q")
            nc.vector.memset(ones_q[:], 1.0)
            ones77r = consts.tile([1, SKV], BF16, tag="ones77r")
            nc.vector.memset(ones77r[:], 1.0)
            ones1 = consts.tile([1, 128], BF16, tag="ones1")
            nc.vector.memset(ones1[:], 1.0)
            sel16_sb = consts.tile([SKV, H * 16], BF16, tag="sel16")
            nc.sync.dma_start(sel16_sb[:], sel16_d.ap())
            sel64_sb = consts.tile([16, H * DH], BF16, tag="sel64")
            nc.sync.dma_start(sel64_sb[:], sel64_d.ap())

            kT_sb = consts.tile([128, CT, SKV], BF16, tag="kT")
            # filled after v projection: ones column for fused exp-sums
            v_aug = consts.tile([SKV, H, DH + 1], BF16, tag="v")

            # ---- k projection: kT[c, s] = sum_k wk[k, c] yT[k, s]; fold (.+bk)/8 ----
            for ct in range(CT):
                psk = pmm.tile([128, CH], F32, tag="mm")
                for kt in range(KC):
                    nc.tensor.matmul(
                        psk[:, 0:SKV],
                        wk_sb[:, kt, ct * 128:(ct + 1) * 128],
                        yt_sb[:, kt, :],
                        start=(kt == 0),
                        stop=(kt == KC - 1),
                    )
                nc.scalar.activation(
                    kT_sb[:, ct, :],
                    psk[:, 0:SKV],
                    AF.Identity,
                    scale=0.125,
                    bias=bk8_sb[:, ct:ct + 1],
                )

            # ---- v projection: v[s, c] = sum_k yT[k, s] wv[k, c] + bv[c] ----
            for n in range(2):
                psv = pmm.tile([128, CH], F32, tag="mm")
                for kt in range(KC):
                    nc.tensor.matmul(
                        psv[0:SKV, :],
                        yt_sb[:, kt, :],
                        wv_sb[:, kt, n * 512:(n + 1) * 512],
                        start=(kt == 0),
                        stop=False,
                    )
                nc.tensor.matmul(
                    psv[0:SKV, :],
                    ones77r[:],
                    bv_sb[0:1, n * 512:(n + 1) * 512],
                    start=False,
                    stop=True,
                )
                for j in range(8):
                    h = n * 8 + j
                    nc.any.tensor_copy(v_aug[:, h, 0:DH], psv[0:SKV, j * DH:(j + 1) * DH])

            nc.vector.memset(v_aug[:, :, DH:DH + 1], 1.0)

            # ---- main loop over query chunks ----
            for c in range(NCH):
                q0 = c * CH

                xT_ch = xpool.tile([128, KT, CH], BF16, tag="xT")
                nc.sync.dma_start(
                    xT_ch[:],
                    xt.ap().rearrange("(kt p) q -> p kt q", p=128)[:, :, q0:q0 + CH],
                )

                # q^T projection (per column-tile), bias via rank-1 matmul
                qT = qpool.tile([128, CT, CH], BF16, tag="qT")
                for ct in range(CT):
                    psq = pmm.tile([128, CH], F32, tag="mm")
                    for kt in range(KT):
                        nc.tensor.matmul(
                            psq[:],
                            wq_sb[:, kt, ct * 128:(ct + 1) * 128],
                            xT_ch[:, kt, :],
                            start=(kt == 0),
                            stop=False,
                        )
                    nc.tensor.matmul(
                        psq[:],
                        bq_sb[0:1, ct * 128:(ct + 1) * 128],
                        ones_q[:],
                        start=False,
                        stop=True,
                    )
                    nc.vector.tensor_copy(qT[:, ct, :], psq[:])

                # attention pass A: scores -> exp -> sum-collect [16, CH]
                e_ch = epool.tile([SKV, H, CH], BF16, tag="e")
                ps_sum = pnrm.tile([16, CH], F32, tag="nrm")
                for h in range(H):
                    pssc = psc.tile([SKV, CH], F32, tag="sc")
                    nc.tensor.matmul(
                        pssc[:],
                        kT_sb[(h % 2) * 64:(h % 2) * 64 + 64, h // 2, :],
                        qT[(h % 2) * 64:(h % 2) * 64 + 64, h // 2, :],
                        start=True, stop=True,
                    )
                    nc.scalar.activation(e_ch[:, h, :], pssc[:], AF.Exp)
                    nc.tensor.matmul(
                        ps_sum[:], sel16_sb[:, h * 16:(h + 1) * 16], e_ch[:, h, :],
                        start=(h == 0), stop=(h == H - 1), skip_group_check=True,
                    )
                r16 = rp.tile([16, CH], BF16, tag="r")
                with nc.allow_low_precision(reason="softmax recip in bf16"):
                    nc.vector.reciprocal(r16[:], ps_sum[:])

                # pass B: broadcast 1/sum, PV, normalize
                aoT = aopool.tile([128, KT, CH], BF16, tag="aoT")
                for hp in range(H // 2):
                    rb_ps = pnrm.tile([128, CH], F32, tag="nrm")
                    for half in range(2):
                        h = 2 * hp + half
                        nc.tensor.matmul(
                            rb_ps[half * 64:(half + 1) * 64, :],
                            sel64_sb[:, h * 64:(h + 1) * 64],
                            r16[:],
                            start=True, stop=True,
                        )
                    rb_sb = rbpool.tile([128, CH], F32, tag="rb")
                    nc.scalar.activation(rb_sb[:], rb_ps[:], AF.Identity)
                    pspv = ppv.tile([128, CH], F32, tag="pv")
                    for half in range(2):
                        h = 2 * hp + half
                        nc.tensor.matmul(
                            pspv[half * 64:(half + 1) * 64, :],
                            v_aug[:, h, 0:DH],
                            e_ch[:, h, :],
                            start=True, stop=True,
                        )
                    for half in range(2):
                        nc.vector.tensor_mul(
                            aoT[half * 64:(half + 1) * 64, hp, :],
                            pspv[half * 64:(half + 1) * 64, :],
                            rb_sb[half * 64:(half + 1) * 64, :],
                        )

                # output projection: out[q, n] = sum_kt aoT[kt, q]^T wo[kt, n] + bo
                for qt in range(NQT):
                    for n in range(2):
                        pso = pmm.tile([128, 512], F32, tag="mm")
                        for kt in range(KT):
                            nc.tensor.matmul(
                                pso[:],
                                aoT[:, kt, qt * 128:(qt + 1) * 128],
                                wo_sb[:, kt, n * 512:(n + 1) * 512],
                                start=(kt == 0),
                                stop=False,
                            )
                        nc.tensor.matmul(
                            pso[:],
                            ones1[:],
                            bo_sb[0:1, n * 512:(n + 1) * 512],
                            start=False,
                            stop=True,
                        )
                        o_sb = opool.tile([128, 512], F32, tag="o")
                        nc.vector.tensor_copy(o_sb[:], pso[:])
                        nc.sync.dma_start(
                            out_d.ap()[q0 + qt * 128: q0 + (qt + 1) * 128,
                                       n * 512:(n + 1) * 512],
                            o_sb[:],
                        )

    nc.compile()
    return nc


def _get_nc():
    if "nc" not in _CACHED:
        _CACHED["nc"] = _build()
    return _CACHED["nc"]


def kernel(x, y, wq, bq, wk, bk, wv, bv, wo, bo):
    x = np.asarray(x)
    y = np.asarray(y)
    bf = ml_dtypes.bfloat16
    wq_b = np.asarray(wq).astype(bf)
    wk_b = np.asarray(wk).astype(bf)
    wv_b = np.asarray(wv).astype(bf)
    wo_b = np.asarray(wo).astype(bf)
    bq_b = np.asarray(bq).reshape(1, D).astype(bf)
    bv_b = np.asarray(bv).reshape(1, D).astype(bf)
    bo_b = np.asarray(bo).reshape(1, D).astype(bf)
    bk8 = (np.asarray(bk).reshape(1, D) * 0.125).astype(np.float32)
    sel16 = np.zeros((SKV, H, 16), np.float32)
    sel16[:, np.arange(H), np.arange(16)] = 1.0
    sel16 = sel16.reshape(SKV, H * 16).astype(bf)
    sel64 = np.zeros((16, H, DH), np.float32)
    sel64[np.arange(16), np.arange(H), :] = 1.0
    sel64 = sel64.reshape(16, H * DH).astype(bf)

    in_maps = []
    for b in range(B):
        in_maps.append({
            "xt": np.ascontiguousarray(x[b].T).astype(bf),
            "yt": np.ascontiguousarray(y[b].T).astype(bf),
            "wq": wq_b, "wk": wk_b, "wv": wv_b, "wo": wo_b,
            "bq": bq_b, "bk8": bk8, "bv": bv_b, "bo": bo_b,
            "sel16": sel16, "sel64": sel64,
        })

    nc = _get_nc()
    res = bass_utils.run_bass_kernel_spmd(nc, in_maps, core_ids=list(range(B)))
    out = np.stack([res.results[b]["out"] for b in range(B)], axis=0)
    return out.astype(np.float32)



# revision 4
# speedup vs baseline: 1.2982x; 1.2982x over previous
"""Cross-attention kernel for TRN2, SPMD over 8 NeuronCores.

Problem: B=8, SQ=4096, SKV=77, D_EMBED=1024, D_CROSS=768, H=16, DH=64.
  q = x @ wq + bq ; k = y @ wk + bk ; v = y @ wv + bv
  out = softmax(q k^T / 8) v @ wo + bo

Sharding: pure data-parallel over batch (1 batch element per core, no
collectives). Host pre-transposes x and y per core so the device kernel
keeps every tensor feature-major (contraction dim on partitions) until the
O-projection, which uses attnout^T as the stationary operand to emit the
output in natural row-major layout.

Perf structure (v2): the PE instruction stream is kept dense so the HAM
clock-gate stays at 8/8 (2.4 GHz) instead of oscillating:
  - The O-projection of chunk c-1 is interleaved into the ACT-bound
    attention phase of chunk c (software pipeline, aoT double-buffered).
  - Bias adds no longer burn PE matmuls: q-bias via per-partition bias on
    the scalar-engine PSUM->SBUF copy, o-bias via a DVE tensor_add against
    a pre-broadcast [128, D] bias tile.
  - The 1/sum broadcast uses one [16,128]-selector matmul per head pair
    (half the baseline's matmul count for this step).

Compute dtype: bf16 operands (host-cast), fp32 PSUM accumulation, fp32 out.

Softmax is computed without max-subtraction (scores are O(5) for this
problem class; exp stays comfortably inside fp32/bf16 range):
  scoresT[s,q] = k'_h @ q_h^T with k' = (k + bk)/8 folded at k-projection
  e = exp(scoresT)  (bf16)
  r = 1 / (sel16^T @ e)         per-head [16, SQ] via PE column-sum
  aoT[d,q] = (v_h^T @ e) * rb   with rb = selb^T @ r (PE broadcast)
  out[q,:] = aoT^T @ wo + bo    (aoT tiles as stationary operand)
"""

import numpy as np
import ml_dtypes

import concourse.bass as bass
import concourse.mybir as mybir
import concourse.tile as tile
from concourse import bacc
from concourse import bass_utils

F32 = mybir.dt.float32
BF16 = mybir.dt.bfloat16
AF = mybir.ActivationFunctionType

B = 8
SQ = 4096
SKV = 77
D = 1024
DC = 768
H = 16
DH = 64
KT = D // 128    # 8 embed k-tiles
KC = DC // 128   # 6 cross k-tiles
CT = D // 128    # 8 column tiles of the 1024-wide projections
CH = 512         # query chunk
NCH = SQ // CH   # 8 chunks
NQT = CH // 128  # 4 query 128-tiles per chunk

_CACHED = {}


def _build():
    nc = bacc.Bacc("TRN2", target_bir_lowering=False, debug=False, num_devices=B)

    xt = nc.dram_tensor("xt", (D, SQ), BF16, kind="ExternalInput")
    yt = nc.dram_tensor("yt", (DC, SKV), BF16, kind="ExternalInput")
    wq_d = nc.dram_tensor("wq", (D, D), BF16, kind="ExternalInput")
    wk_d = nc.dram_tensor("wk", (DC, D), BF16, kind="ExternalInput")
    wv_d = nc.dram_tensor("wv", (DC, D), BF16, kind="ExternalInput")
    wo_d = nc.dram_tensor("wo", (D, D), BF16, kind="ExternalInput")
    bqc_d = nc.dram_tensor("bqc", (1, D), F32, kind="ExternalInput")
    bk8_d = nc.dram_tensor("bk8", (1, D), F32, kind="ExternalInput")
    bv_d = nc.dram_tensor("bv", (1, D), BF16, kind="ExternalInput")
    bo_d = nc.dram_tensor("bo", (1, D), BF16, kind="ExternalInput")
    sel16_d = nc.dram_tensor("sel16", (SKV, H * 16), BF16, kind="ExternalInput")
    selb_d = nc.dram_tensor("selb", (16, D), BF16, kind="ExternalInput")
    out_d = nc.dram_tensor("out", (SQ, D), F32, kind="ExternalOutput")

    with tile.TileContext(nc) as tc:
        with (
            tc.tile_pool(name="consts", bufs=1) as consts,
            tc.tile_pool(name="wpool", bufs=1) as wpool,
            tc.tile_pool(name="xpool", bufs=2) as xpool,
            tc.tile_pool(name="qpool", bufs=2) as qpool,
            tc.tile_pool(name="epool", bufs=2) as epool,
            tc.tile_pool(name="rp", bufs=2) as rp,
            tc.tile_pool(name="rbpool", bufs=2) as rbpool,
            tc.tile_pool(name="aopool", bufs=2) as aopool,
            tc.tile_pool(name="opool", bufs=3) as opool,
            tc.tile_pool(name="pmm", bufs=2, space="PSUM") as pmm,
            tc.tile_pool(name="psc", bufs=2, space="PSUM") as psc,
            tc.tile_pool(name="ppv", bufs=2, space="PSUM") as ppv,
            tc.tile_pool(name="pnrm", bufs=2, space="PSUM") as pnrm,
        ):
            # ---- constants / weights ----
            wq_sb = wpool.tile([128, KT, D], BF16, tag="wq")
            nc.sync.dma_start(wq_sb[:], wq_d.ap().rearrange("(kt p) n -> p kt n", p=128))
            wo_sb = wpool.tile([128, KT, D], BF16, tag="wo")
            nc.sync.dma_start(wo_sb[:], wo_d.ap().rearrange("(kt p) n -> p kt n", p=128))
            wk_sb = wpool.tile([128, KC, D], BF16, tag="wk")
            nc.sync.dma_start(wk_sb[:], wk_d.ap().rearrange("(kt p) n -> p kt n", p=128))
            wv_sb = wpool.tile([128, KC, D], BF16, tag="wv")
            nc.sync.dma_start(wv_sb[:], wv_d.ap().rearrange("(kt p) n -> p kt n", p=128))

            yt_sb = consts.tile([128, KC, SKV], BF16, tag="yt")
            nc.sync.dma_start(yt_sb[:], yt.ap().rearrange("(kt p) s -> p kt s", p=128))

            bv_sb = consts.tile([1, D], BF16, tag="bv")
            nc.sync.dma_start(bv_sb[:], bv_d.ap())
            bo_sb = consts.tile([1, D], BF16, tag="bo")
            nc.sync.dma_start(bo_sb[:], bo_d.ap())
            bk8_sb = consts.tile([128, CT], F32, tag="bk8")
            nc.sync.dma_start(bk8_sb[:], bk8_d.ap().rearrange("a (ct p) -> (a p) ct", p=128))
            bqc_sb = consts.tile([128, CT], F32, tag="bqc")
            nc.sync.dma_start(bqc_sb[:], bqc_d.ap().rearrange("a (ct p) -> (a p) ct", p=128))

            ones77r = consts.tile([1, SKV], BF16, tag="ones77r")
            nc.vector.memset(ones77r[:], 1.0)
            ones1 = consts.tile([1, 128], BF16, tag="ones1")
            nc.vector.memset(ones1[:], 1.0)
            sel16_sb = consts.tile([SKV, H * 16], BF16, tag="sel16")
            nc.sync.dma_start(sel16_sb[:], sel16_d.ap())
            selb_sb = consts.tile([16, D], BF16, tag="selb")
            nc.sync.dma_start(selb_sb[:], selb_d.ap())

            kT_sb = consts.tile([128, CT, SKV], BF16, tag="kT")
            v_sb = consts.tile([SKV, H, DH], BF16, tag="v")
            bo128 = consts.tile([128, D], BF16, tag="bo128")

            # ---- bo broadcast to all 128 partitions (one-time, 2 matmuls) ----
            for n in range(2):
                psb = pmm.tile([128, CH], F32, tag="mm")
                nc.tensor.matmul(
                    psb[:],
                    ones1[0:1, :],
                    bo_sb[0:1, n * 512:(n + 1) * 512],
                    start=True, stop=True,
                )
                nc.vector.tensor_copy(bo128[:, n * 512:(n + 1) * 512], psb[:])

            # ---- k projection: kT[c, s] = sum_k wk[k, c] yT[k, s]; fold (.+bk)/8 ----
            for ct in range(CT):
                psk = pmm.tile([128, CH], F32, tag="mm")
                for kt in range(KC):
                    nc.tensor.matmul(
                        psk[:, 0:SKV],
                        wk_sb[:, kt, ct * 128:(ct + 1) * 128],
                        yt_sb[:, kt, :],
                        start=(kt == 0),
                        stop=(kt == KC - 1),
                    )
                nc.scalar.activation(
                    kT_sb[:, ct, :],
                    psk[:, 0:SKV],
                    AF.Identity,
                    scale=0.125,
                    bias=bk8_sb[:, ct:ct + 1],
                )

            # ---- v projection: v[s, c] = sum_k yT[k, s] wv[k, c] + bv[c] ----
            for n in range(2):
                psv = pmm.tile([128, CH], F32, tag="mm")
                for kt in range(KC):
                    nc.tensor.matmul(
                        psv[0:SKV, :],
                        yt_sb[:, kt, :],
                        wv_sb[:, kt, n * 512:(n + 1) * 512],
                        start=(kt == 0),
                        stop=False,
                    )
                nc.tensor.matmul(
                    psv[0:SKV, :],
                    ones77r[:],
                    bv_sb[0:1, n * 512:(n + 1) * 512],
                    start=False,
                    stop=True,
                )
                nc.vector.tensor_copy(
                    v_sb[:, n * 8:(n + 1) * 8, :], psv[0:SKV, :]
                )

            # ---- software-pipelined main loop ----
            # iteration c: Q-proj(c) + attention(c), with O-proj(c-1) tiles
            # interleaved into the PE stream to keep it dense while ACT/DVE
            # work on softmax.
            aoT_tiles = [None, None]

            def emit_op_tile(cc, t):
                """O-proj tile t=(qt*2+n) of chunk cc: 8 matmuls + DVE bias-add + DMA."""
                qt, n = t // 2, t % 2
                aoT_p = aoT_tiles[cc % 2]
                q0 = cc * CH
                pso = pmm.tile([128, CH], F32, tag="mm")
                for kt in range(KT):
                    nc.tensor.matmul(
                        pso[:],
                        aoT_p[:, kt, qt * 128:(qt + 1) * 128],
                        wo_sb[:, kt, n * 512:(n + 1) * 512],
                        start=(kt == 0),
                        stop=(kt == KT - 1),
                        skip_group_check=True,
                    )
                o_sb = opool.tile([128, CH], F32, tag="o")
                nc.vector.tensor_add(o_sb[:], pso[:], bo128[:, n * 512:(n + 1) * 512])
                nc.sync.dma_start(
                    out_d.ap()[q0 + qt * 128: q0 + (qt + 1) * 128,
                               n * 512:(n + 1) * 512],
                    o_sb[:],
                )

            for c in range(NCH + 1):
                op_t = [0]  # next O-proj tile index for chunk c-1

                def drain_op(upto, _c=c, _op_t=op_t):
                    if _c == 0:
                        return
                    while _op_t[0] < upto:
                        emit_op_tile(_c - 1, _op_t[0])
                        _op_t[0] += 1

                if c == NCH:
                    drain_op(8)
                    break

                q0 = c * CH
                xT_ch = xpool.tile([128, KT, CH], BF16, tag="xT")
                nc.sync.dma_start(
                    xT_ch[:],
                    xt.ap().rearrange("(kt p) q -> p kt q", p=128)[:, :, q0:q0 + CH],
                )

                # q^T projection (per column-tile); bias via per-partition
                # activation bias on the PSUM->SBUF copy (no PE matmul).
                qT = qpool.tile([128, CT, CH], BF16, tag="qT")
                for ct in range(CT):
                    psq = pmm.tile([128, CH], F32, tag="mm")
                    for kt in range(KT):
                        nc.tensor.matmul(
                            psq[:],
                            wq_sb[:, kt, ct * 128:(ct + 1) * 128],
                            xT_ch[:, kt, :],
                            start=(kt == 0),
                            stop=(kt == KT - 1),
                            skip_group_check=True,
                        )
                    nc.scalar.activation(
                        qT[:, ct, :],
                        psq[:],
                        AF.Identity,
                        bias=bqc_sb[:, ct:ct + 1],
                    )

                # attention pass A: scores -> exp -> sum-collect [16, CH]
                e_ch = epool.tile([SKV, H, CH], BF16, tag="e")
                ps_sum = pnrm.tile([16, CH], F32, tag="nrm")
                for h in range(H):
                    pssc = psc.tile([SKV, CH], F32, tag="sc")
                    nc.tensor.matmul(
                        pssc[:],
                        kT_sb[(h % 2) * 64:(h % 2) * 64 + 64, h // 2, :],
                        qT[(h % 2) * 64:(h % 2) * 64 + 64, h // 2, :],
                        start=True, stop=True, skip_group_check=True,
                    )
                    nc.scalar.activation(e_ch[:, h, :], pssc[:], AF.Exp)
                    nc.tensor.matmul(
                        ps_sum[:], sel16_sb[:, h * 16:(h + 1) * 16], e_ch[:, h, :],
                        start=(h == 0), stop=(h == H - 1), skip_group_check=True,
                    )
                    if h in (2, 5, 8, 11, 14):
                        drain_op({2: 1, 5: 2, 8: 3, 11: 4, 14: 5}[h])

                r16 = rp.tile([16, CH], BF16, tag="r")
                with nc.allow_low_precision(reason="softmax recip in bf16"):
                    nc.vector.reciprocal(r16[:], ps_sum[:])
                drain_op(6)

                # pass B: rb = selb^T @ r (broadcast 1/sum to 128 rows), PV,
                # normalize into aoT.
                aoT = aopool.tile([128, KT, CH], BF16, tag="aoT")
                aoT_tiles[c % 2] = aoT
                for hp in range(H // 2):
                    rb_ps = pnrm.tile([128, CH], F32, tag="nrm")
                    nc.tensor.matmul(
                        rb_ps[:],
                        selb_sb[:, hp * 128:(hp + 1) * 128],
                        r16[:],
                        start=True, stop=True, skip_group_check=True,
                    )
                    rb_sb = rbpool.tile([128, CH], F32, tag="rb")
                    nc.scalar.activation(rb_sb[:], rb_ps[:], AF.Identity)
                    pspv = ppv.tile([128, CH], F32, tag="pv")
                    for half in range(2):
                        h = 2 * hp + half
                        nc.tensor.matmul(
                            pspv[half * 64:(half + 1) * 64, :],
                            v_sb[:, h, :],
                            e_ch[:, h, :],
                            start=True, stop=True, skip_group_check=True,
                        )
                    nc.vector.tensor_mul(aoT[:, hp, :], pspv[:], rb_sb[:])
                    if hp in (2, 5):
                        drain_op({2: 7, 5: 8}[hp])

    nc.compile()
    return nc


def _get_nc():
    if "nc" not in _CACHED:
        _CACHED["nc"] = _build()
    return _CACHED["nc"]


def _prep_inmaps(x, y, wq, bq, wk, bk, wv, bv, wo, bo):
    x = np.asarray(x)
    y = np.asarray(y)
    bf = ml_dtypes.bfloat16
    wq_b = np.asarray(wq).astype(bf)
    wk_b = np.asarray(wk).astype(bf)
    wv_b = np.asarray(wv).astype(bf)
    wo_b = np.asarray(wo).astype(bf)
    bv_b = np.asarray(bv).reshape(1, D).astype(bf)
    bo_b = np.asarray(bo).reshape(1, D).astype(bf)
    bqc = np.asarray(bq).reshape(1, D).astype(np.float32)
    bk8 = (np.asarray(bk).reshape(1, D) * 0.125).astype(np.float32)
    sel16 = np.zeros((SKV, H, 16), np.float32)
    sel16[:, np.arange(H), np.arange(16)] = 1.0
    sel16 = sel16.reshape(SKV, H * 16).astype(bf)
    # selb[i, hp*128 + p] = 1 iff i == 2*hp + (p >= 64)
    selb = np.zeros((16, H // 2, 128), np.float32)
    for hp in range(H // 2):
        selb[2 * hp, hp, 0:64] = 1.0
        selb[2 * hp + 1, hp, 64:128] = 1.0
    selb = selb.reshape(16, D).astype(bf)

    in_maps = []
    for b in range(B):
        in_maps.append({
            "xt": np.ascontiguousarray(x[b].T).astype(bf),
            "yt": np.ascontiguousarray(y[b].T).astype(bf),
            "wq": wq_b, "wk": wk_b, "wv": wv_b, "wo": wo_b,
            "bqc": bqc, "bk8": bk8, "bv": bv_b, "bo": bo_b,
            "sel16": sel16, "selb": selb,
        })
    return in_maps


def kernel(x, y, wq, bq, wk, bk, wv, bv, wo, bo):
    in_maps = _prep_inmaps(x, y, wq, bq, wk, bk, wv, bv, wo, bo)
    nc = _get_nc()
    res = bass_utils.run_bass_kernel_spmd(nc, in_maps, core_ids=list(range(B)))
    out = np.stack([res.results[b]["out"] for b in range(B)], axis=0)
    return out.astype(np.float32)


# revision 12
# speedup vs baseline: 1.3837x; 1.0659x over previous
"""Cross-attention kernel for TRN2, SPMD over 8 NeuronCores.

Problem: B=8, SQ=4096, SKV=77, D_EMBED=1024, D_CROSS=768, H=16, DH=64.
  q = x @ wq + bq ; k = y @ wk + bk ; v = y @ wv + bv
  out = softmax(q k^T / 8) v @ wo + bo

Sharding: pure data-parallel over batch (1 batch element per core, no
collectives). Host pre-transposes x and y per core so the device kernel
keeps every tensor feature-major (contraction dim on partitions) until the
O-projection, which uses attnout^T as the stationary operand to emit the
output in natural row-major layout.

Perf structure (v2): the PE instruction stream is kept dense so the HAM
clock-gate stays at 8/8 (2.4 GHz) instead of oscillating:
  - The O-projection of chunk c-1 is interleaved into the ACT-bound
    attention phase of chunk c (software pipeline, aoT double-buffered).
  - Bias adds no longer burn PE matmuls: q-bias via per-partition bias on
    the scalar-engine PSUM->SBUF copy, o-bias via a DVE tensor_add against
    a pre-broadcast [128, D] bias tile.
  - The 1/sum broadcast uses one [16,128]-selector matmul per head pair
    (half the baseline's matmul count for this step).

Compute dtype: bf16 operands (host-cast), fp32 PSUM accumulation, fp32 out.

Softmax is computed without max-subtraction (scores are O(5) for this
problem class; exp stays comfortably inside fp32/bf16 range):
  scoresT[s,q] = k'_h @ q_h^T with k' = (k + bk)/8 folded at k-projection
  e = exp(scoresT)  (bf16)
  r = 1 / (sel16^T @ e)         per-head [16, SQ] via PE column-sum
  aoT[d,q] = (v_h^T @ e) * rb   with rb = selb^T @ r (PE broadcast)
  out[q,:] = aoT^T @ wo + bo    (aoT tiles as stationary operand)
"""

import numpy as np
import ml_dtypes

import concourse.bass as bass
import concourse.mybir as mybir
import concourse.tile as tile
from concourse import bacc
from concourse import bass_utils

F32 = mybir.dt.float32
BF16 = mybir.dt.bfloat16
AF = mybir.ActivationFunctionType

B = 8
SQ = 4096
SKV = 77
D = 1024
DC = 768
H = 16
DH = 64
KT = D // 128    # 8 embed k-tiles
KC = DC // 128   # 6 cross k-tiles
CT = D // 128    # 8 column tiles of the 1024-wide projections
CH = 512         # query chunk
NCH = SQ // CH   # 8 chunks
NQT = CH // 128  # 4 query 128-tiles per chunk

_CACHED = {}


def _build():
    nc = bacc.Bacc("TRN2", target_bir_lowering=False, debug=False, num_devices=B)

    xt = nc.dram_tensor("xt", (D, SQ), BF16, kind="ExternalInput")
    yt = nc.dram_tensor("yt", (DC, SKV), BF16, kind="ExternalInput")
    wq_d = nc.dram_tensor("wq", (D, D), BF16, kind="ExternalInput")
    wk_d = nc.dram_tensor("wk", (DC, D), BF16, kind="ExternalInput")
    wv_d = nc.dram_tensor("wv", (DC, D), BF16, kind="ExternalInput")
    wo_d = nc.dram_tensor("wo", (D, D), BF16, kind="ExternalInput")
    bqc_d = nc.dram_tensor("bqc", (1, D), F32, kind="ExternalInput")
    bk8_d = nc.dram_tensor("bk8", (1, D), F32, kind="ExternalInput")
    bv_d = nc.dram_tensor("bv", (1, D), BF16, kind="ExternalInput")
    bo128_d = nc.dram_tensor("bo128", (128, D), BF16, kind="ExternalInput")
    sel16_d = nc.dram_tensor("sel16", (SKV, H * 16), BF16, kind="ExternalInput")
    selb_d = nc.dram_tensor("selb", (16, D), BF16, kind="ExternalInput")
    out_d = nc.dram_tensor("out", (SQ, D), F32, kind="ExternalOutput")

    with tile.TileContext(nc) as tc:
        with (
            tc.tile_pool(name="consts", bufs=1) as consts,
            tc.tile_pool(name="wpool", bufs=1) as wpool,
            tc.tile_pool(name="xpool", bufs=2) as xpool,
            tc.tile_pool(name="qpool", bufs=2) as qpool,
            tc.tile_pool(name="epool", bufs=2) as epool,
            tc.tile_pool(name="rp", bufs=2) as rp,
            tc.tile_pool(name="rbpool", bufs=2) as rbpool,
            tc.tile_pool(name="aopool", bufs=2) as aopool,
            tc.tile_pool(name="opool", bufs=3) as opool,
            tc.tile_pool(name="pmm", bufs=2, space="PSUM") as pmm,
            tc.tile_pool(name="psc", bufs=2, space="PSUM") as psc,
            tc.tile_pool(name="ppv", bufs=2, space="PSUM") as ppv,
            tc.tile_pool(name="pnrm", bufs=2, space="PSUM") as pnrm,
        ):
            # ---- constants / weights (ordered so K/V-proj inputs land first,
            # then wq for Q-proj(0); wo is not needed until chunk 1) ----
            yt_sb = consts.tile([128, KC, SKV], BF16, tag="yt")
            nc.sync.dma_start(yt_sb[:], yt.ap().rearrange("(kt p) s -> p kt s", p=128))
            wk_sb = wpool.tile([128, KC, D], BF16, tag="wk")
            nc.sync.dma_start(wk_sb[:], wk_d.ap().rearrange("(kt p) n -> p kt n", p=128))
            wv_sb = wpool.tile([128, KC, D], BF16, tag="wv")
            nc.sync.dma_start(wv_sb[:], wv_d.ap().rearrange("(kt p) n -> p kt n", p=128))
            wq_sb = wpool.tile([128, KT, D], BF16, tag="wq")
            nc.sync.dma_start(wq_sb[:], wq_d.ap().rearrange("(kt p) n -> p kt n", p=128))

            bv_sb = consts.tile([1, D], BF16, tag="bv")
            nc.sync.dma_start(bv_sb[:], bv_d.ap())
            bk8_sb = consts.tile([128, CT], F32, tag="bk8")
            nc.sync.dma_start(bk8_sb[:], bk8_d.ap().rearrange("a (ct p) -> (a p) ct", p=128))
            bqc_sb = consts.tile([128, CT], F32, tag="bqc")
            nc.sync.dma_start(bqc_sb[:], bqc_d.ap().rearrange("a (ct p) -> (a p) ct", p=128))

            ones77r = consts.tile([1, SKV], BF16, tag="ones77r")
            nc.vector.memset(ones77r[:], 1.0)
            sel16_sb = consts.tile([SKV, H * 16], BF16, tag="sel16")
            nc.sync.dma_start(sel16_sb[:], sel16_d.ap())
            selb_sb = consts.tile([16, D], BF16, tag="selb")
            nc.sync.dma_start(selb_sb[:], selb_d.ap())
            bo128 = consts.tile([128, D], BF16, tag="bo128")
            nc.sync.dma_start(bo128[:], bo128_d.ap())
            wo_sb = wpool.tile([128, KT, D], BF16, tag="wo")
            nc.sync.dma_start(wo_sb[:], wo_d.ap().rearrange("(kt p) n -> p kt n", p=128))

            kT_sb = consts.tile([128, CT, SKV], BF16, tag="kT")
            v_sb = consts.tile([SKV, H, DH], BF16, tag="v")

            # ---- k projection: kT[c, s] = sum_k wk[k, c] yT[k, s]; fold (.+bk)/8 ----
            for ct in range(CT):
                psk = pmm.tile([128, CH], F32, tag="mm")
                for kt in range(KC):
                    nc.tensor.matmul(
                        psk[:, 0:SKV],
                        wk_sb[:, kt, ct * 128:(ct + 1) * 128],
                        yt_sb[:, kt, :],
                        start=(kt == 0),
                        stop=(kt == KC - 1),
                    )
                nc.scalar.activation(
                    kT_sb[:, ct, :],
                    psk[:, 0:SKV],
                    AF.Identity,
                    scale=0.125,
                    bias=bk8_sb[:, ct:ct + 1],
                )

            # ---- v projection: v[s, c] = sum_k yT[k, s] wv[k, c] + bv[c] ----
            for n in range(2):
                psv = pmm.tile([128, CH], F32, tag="mm")
                for kt in range(KC):
                    nc.tensor.matmul(
                        psv[0:SKV, :],
                        yt_sb[:, kt, :],
                        wv_sb[:, kt, n * 512:(n + 1) * 512],
                        start=(kt == 0),
                        stop=False,
                    )
                nc.tensor.matmul(
                    psv[0:SKV, :],
                    ones77r[:],
                    bv_sb[0:1, n * 512:(n + 1) * 512],
                    start=False,
                    stop=True,
                )
                nc.vector.tensor_copy(
                    v_sb[:, n * 8:(n + 1) * 8, :], psv[0:SKV, :]
                )

            # ---- software-pipelined main loop ----
            # iteration c: Q-proj(c) + attention(c), with O-proj(c-1) tiles
            # interleaved into the PE stream to keep it dense while ACT/DVE
            # work on softmax.
            aoT_tiles = [None, None]
            xT_tiles = {}

            def fetch_x(cc):
                if cc >= NCH:
                    return
                t = xpool.tile([128, KT, CH], BF16, tag="xT")
                nc.sync.dma_start(
                    t[:],
                    xt.ap().rearrange("(kt p) q -> p kt q", p=128)[
                        :, :, cc * CH:(cc + 1) * CH],
                )
                xT_tiles[cc] = t

            def emit_op_tile(cc, t):
                """O-proj tile t=(qt*2+n) of chunk cc: 8 matmuls + DVE bias-add + DMA."""
                qt, n = t // 2, t % 2
                aoT_p = aoT_tiles[cc % 2]
                q0 = cc * CH
                pso = pmm.tile([128, CH], F32, tag="mm")
                for kt in range(KT):
                    nc.tensor.matmul(
                        pso[:],
                        aoT_p[:, kt, qt * 128:(qt + 1) * 128],
                        wo_sb[:, kt, n * 512:(n + 1) * 512],
                        start=(kt == 0),
                        stop=(kt == KT - 1),
                        skip_group_check=True,
                    )
                o_sb = opool.tile([128, CH], F32, tag="o")
                nc.vector.tensor_add(o_sb[:], pso[:], bo128[:, n * 512:(n + 1) * 512])
                nc.sync.dma_start(
                    out_d.ap()[q0 + qt * 128: q0 + (qt + 1) * 128,
                               n * 512:(n + 1) * 512],
                    o_sb[:],
                )

            for c in range(NCH + 1):
                op_t = [0]  # next O-proj tile index for chunk c-1

                def drain_op(upto, _c=c, _op_t=op_t):
                    if _c == 0:
                        return
                    while _op_t[0] < upto:
                        emit_op_tile(_c - 1, _op_t[0])
                        _op_t[0] += 1

                if c == NCH:
                    drain_op(8)
                    break

                if c == 0:
                    fetch_x(0)
                fetch_x(c + 1)
                xT_ch = xT_tiles.pop(c)

                # q^T projection (per column-tile); bias via per-partition
                # activation bias on the PSUM->SBUF copy (no PE matmul).
                qT = qpool.tile([128, CT, CH], BF16, tag="qT")
                for ct in range(CT):
                    psq = pmm.tile([128, CH], F32, tag="mm")
                    for kt in range(KT):
                        nc.tensor.matmul(
                            psq[:],
                            wq_sb[:, kt, ct * 128:(ct + 1) * 128],
                            xT_ch[:, kt, :],
                            start=(kt == 0),
                            stop=(kt == KT - 1),
                            skip_group_check=True,
                        )
                    nc.scalar.activation(
                        qT[:, ct, :],
                        psq[:],
                        AF.Identity,
                        bias=bqc_sb[:, ct:ct + 1],
                    )

                # attention pass A: scores -> exp -> sum-collect [16, CH]
                e_ch = epool.tile([SKV, H, CH], BF16, tag="e")
                ps_sum = pnrm.tile([16, CH], F32, tag="nrm")
                for h in range(H):
                    pssc = psc.tile([SKV, CH], F32, tag="sc")
                    nc.tensor.matmul(
                        pssc[:],
                        kT_sb[(h % 2) * 64:(h % 2) * 64 + 64, h // 2, :],
                        qT[(h % 2) * 64:(h % 2) * 64 + 64, h // 2, :],
                        start=True, stop=True, skip_group_check=True,
                    )
                    nc.scalar.activation(e_ch[:, h, :], pssc[:], AF.Exp)
                    nc.tensor.matmul(
                        ps_sum[:], sel16_sb[:, h * 16:(h + 1) * 16], e_ch[:, h, :],
                        start=(h == 0), stop=(h == H - 1), skip_group_check=True,
                    )
                    if h in (3, 7, 11):
                        drain_op({3: 1, 7: 2, 11: 3}[h])

                r16 = rp.tile([16, CH], BF16, tag="r")
                with nc.allow_low_precision(reason="softmax recip in bf16"):
                    nc.vector.reciprocal(r16[:], ps_sum[:])
                # 4 O-proj tiles bridge the reciprocal's DVE latency so the
                # PE never idles long enough to trip the HAM re-throttle.
                drain_op(7)

                # pass B: rb = selb^T @ r (broadcast 1/sum to 128 rows), PV,
                # normalize into aoT.
                aoT = aopool.tile([128, KT, CH], BF16, tag="aoT")
                aoT_tiles[c % 2] = aoT
                for hp in range(H // 2):
                    rb_ps = pnrm.tile([128, CH], F32, tag="nrm")
                    nc.tensor.matmul(
                        rb_ps[:],
                        selb_sb[:, hp * 128:(hp + 1) * 128],
                        r16[:],
                        start=True, stop=True, skip_group_check=True,
                    )
                    rb_sb = rbpool.tile([128, CH], F32, tag="rb")
                    nc.scalar.activation(rb_sb[:], rb_ps[:], AF.Identity)
                    pspv = ppv.tile([128, CH], F32, tag="pv")
                    for half in range(2):
                        h = 2 * hp + half
                        nc.tensor.matmul(
                            pspv[half * 64:(half + 1) * 64, :],
                            v_sb[:, h, :],
                            e_ch[:, h, :],
                            start=True, stop=True, skip_group_check=True,
                        )
                    nc.vector.tensor_mul(aoT[:, hp, :], pspv[:], rb_sb[:])
                    if hp == 3:
                        drain_op(8)

    nc.compile()
    return nc


def _get_nc():
    if "nc" not in _CACHED:
        _CACHED["nc"] = _build()
    return _CACHED["nc"]


def _prep_inmaps(x, y, wq, bq, wk, bk, wv, bv, wo, bo):
    x = np.asarray(x)
    y = np.asarray(y)
    bf = ml_dtypes.bfloat16
    wq_b = np.asarray(wq).astype(bf)
    wk_b = np.asarray(wk).astype(bf)
    wv_b = np.asarray(wv).astype(bf)
    wo_b = np.asarray(wo).astype(bf)
    bv_b = np.asarray(bv).reshape(1, D).astype(bf)
    bo128 = np.ascontiguousarray(
        np.broadcast_to(np.asarray(bo).reshape(1, D), (128, D))).astype(bf)
    bqc = np.asarray(bq).reshape(1, D).astype(np.float32)
    bk8 = (np.asarray(bk).reshape(1, D) * 0.125).astype(np.float32)
    sel16 = np.zeros((SKV, H, 16), np.float32)
    sel16[:, np.arange(H), np.arange(16)] = 1.0
    sel16 = sel16.reshape(SKV, H * 16).astype(bf)
    # selb[i, hp*128 + p] = 1 iff i == 2*hp + (p >= 64)
    selb = np.zeros((16, H // 2, 128), np.float32)
    for hp in range(H // 2):
        selb[2 * hp, hp, 0:64] = 1.0
        selb[2 * hp + 1, hp, 64:128] = 1.0
    selb = selb.reshape(16, D).astype(bf)

    in_maps = []
    for b in range(B):
        in_maps.append({
            "xt": np.ascontiguousarray(x[b].T).astype(bf),
            "yt": np.ascontiguousarray(y[b].T).astype(bf),
            "wq": wq_b, "wk": wk_b, "wv": wv_b, "wo": wo_b,
            "bqc": bqc, "bk8": bk8, "bv": bv_b, "bo128": bo128,
            "sel16": sel16, "selb": selb,
        })
    return in_maps


def kernel(x, y, wq, bq, wk, bk, wv, bv, wo, bo):
    in_maps = _prep_inmaps(x, y, wq, bq, wk, bk, wv, bv, wo, bo)
    nc = _get_nc()
    res = bass_utils.run_bass_kernel_spmd(nc, in_maps, core_ids=list(range(B)))
    out = np.stack([res.results[b]["out"] for b in range(B)], axis=0)
    return out.astype(np.float32)


# revision 17
# speedup vs baseline: 1.5151x; 1.0950x over previous
"""Cross-attention kernel for TRN2, SPMD over 8 NeuronCores.

Problem: B=8, SQ=4096, SKV=77, D_EMBED=1024, D_CROSS=768, H=16, DH=64.
  q = x @ wq + bq ; k = y @ wk + bk ; v = y @ wv + bv
  out = softmax(q k^T / 8) v @ wo + bo

Sharding: pure data-parallel over batch (1 batch element per core, no
collectives). Host pre-transposes x and y per core so the device kernel
keeps every tensor feature-major (contraction dim on partitions) until the
O-projection, which uses attnout^T as the stationary operand to emit the
output in natural row-major layout.

Perf structure (v2): the PE instruction stream is kept dense so the HAM
clock-gate stays at 8/8 (2.4 GHz) instead of oscillating:
  - The O-projection of chunk c-1 is interleaved into the ACT-bound
    attention phase of chunk c (software pipeline, aoT double-buffered).
  - Bias adds no longer burn PE matmuls: q-bias via per-partition bias on
    the scalar-engine PSUM->SBUF copy, o-bias via a DVE tensor_add against
    a pre-broadcast [128, D] bias tile.
  - The 1/sum broadcast uses one [16,128]-selector matmul per head pair
    (half the baseline's matmul count for this step).

Compute dtype: bf16 operands (host-cast), fp32 PSUM accumulation, fp32 out.

Softmax is computed without max-subtraction (scores are O(5) for this
problem class; exp stays comfortably inside fp32/bf16 range):
  scoresT[s,q] = k'_h @ q_h^T with k' = (k + bk)/8 folded at k-projection
  e = exp(scoresT)  (bf16)
  r = 1 / (sel16^T @ e)         per-head [16, SQ] via PE column-sum
  aoT[d,q] = (v_h^T @ e) * rb   with rb = selb^T @ r (PE broadcast)
  out[q,:] = aoT^T @ wo + bo    (aoT tiles as stationary operand)
"""

import numpy as np
import ml_dtypes

import concourse.bass as bass
import concourse.mybir as mybir
import concourse.tile as tile
from concourse import bacc
from concourse import bass_utils

F32 = mybir.dt.float32
BF16 = mybir.dt.bfloat16
AF = mybir.ActivationFunctionType

B = 8
SQ = 4096
SKV = 77
D = 1024
DC = 768
H = 16
DH = 64
KT = D // 128    # 8 embed k-tiles
KC = DC // 128   # 6 cross k-tiles
CT = D // 128    # 8 column tiles of the 1024-wide projections
CH = 512         # query chunk
NCH = SQ // CH   # 8 chunks
NQT = CH // 128  # 4 query 128-tiles per chunk

_CACHED = {}


def _build():
    nc = bacc.Bacc("TRN2", target_bir_lowering=False, debug=False, num_devices=B)

    xt = nc.dram_tensor("xt", (D, SQ), BF16, kind="ExternalInput")
    yt = nc.dram_tensor("yt", (DC, SKV), BF16, kind="ExternalInput")
    wq_d = nc.dram_tensor("wq", (D, D), BF16, kind="ExternalInput")
    wk_d = nc.dram_tensor("wk", (DC, D), BF16, kind="ExternalInput")
    wv_d = nc.dram_tensor("wv", (DC, D), BF16, kind="ExternalInput")
    wo_d = nc.dram_tensor("wo", (D, D), BF16, kind="ExternalInput")
    bqc_d = nc.dram_tensor("bqc", (1, D), F32, kind="ExternalInput")
    bk8_d = nc.dram_tensor("bk8", (1, D), F32, kind="ExternalInput")
    bv_d = nc.dram_tensor("bv", (1, D), BF16, kind="ExternalInput")
    bo128_d = nc.dram_tensor("bo128", (128, D), BF16, kind="ExternalInput")
    sel16_d = nc.dram_tensor("sel16", (SKV, H * 16), BF16, kind="ExternalInput")
    selb_d = nc.dram_tensor("selb", (16, D), BF16, kind="ExternalInput")
    out_d = nc.dram_tensor("out", (SQ, D), F32, kind="ExternalOutput")

    with tile.TileContext(nc) as tc:
        with (
            tc.tile_pool(name="consts", bufs=1) as consts,
            tc.tile_pool(name="wpool", bufs=1) as wpool,
            tc.tile_pool(name="xpool", bufs=2) as xpool,
            tc.tile_pool(name="qpool", bufs=2) as qpool,
            tc.tile_pool(name="epool", bufs=2) as epool,
            tc.tile_pool(name="rp", bufs=2) as rp,
            tc.tile_pool(name="rbpool", bufs=2) as rbpool,
            tc.tile_pool(name="aopool", bufs=2) as aopool,
            tc.tile_pool(name="opool", bufs=3) as opool,
            tc.tile_pool(name="pmm", bufs=2, space="PSUM") as pmm,
            tc.tile_pool(name="psc", bufs=2, space="PSUM") as psc,
            tc.tile_pool(name="ppv", bufs=2, space="PSUM") as ppv,
            tc.tile_pool(name="pnrm", bufs=2, space="PSUM") as pnrm,
        ):
            # ---- constants / weights (ordered so K/V-proj inputs land first,
            # then wq for Q-proj(0); wo is not needed until chunk 1) ----
            yt_sb = consts.tile([128, KC, SKV], BF16, tag="yt")
            nc.sync.dma_start(yt_sb[:], yt.ap().rearrange("(kt p) s -> p kt s", p=128))
            wk_sb = wpool.tile([128, KC, D], BF16, tag="wk")
            nc.sync.dma_start(wk_sb[:], wk_d.ap().rearrange("(kt p) n -> p kt n", p=128))
            wv_sb = wpool.tile([128, KC, D], BF16, tag="wv")
            nc.sync.dma_start(wv_sb[:], wv_d.ap().rearrange("(kt p) n -> p kt n", p=128))
            wq_sb = wpool.tile([128, KT, D], BF16, tag="wq")
            nc.sync.dma_start(wq_sb[:], wq_d.ap().rearrange("(kt p) n -> p kt n", p=128))

            bv_sb = consts.tile([1, D], BF16, tag="bv")
            nc.sync.dma_start(bv_sb[:], bv_d.ap())
            bk8_sb = consts.tile([128, CT], F32, tag="bk8")
            nc.sync.dma_start(bk8_sb[:], bk8_d.ap().rearrange("a (ct p) -> (a p) ct", p=128))
            bqc_sb = consts.tile([128, CT], F32, tag="bqc")
            nc.sync.dma_start(bqc_sb[:], bqc_d.ap().rearrange("a (ct p) -> (a p) ct", p=128))

            ones77r = consts.tile([1, SKV], BF16, tag="ones77r")
            nc.vector.memset(ones77r[:], 1.0)
            sel16_sb = consts.tile([SKV, H * 16], BF16, tag="sel16")
            nc.sync.dma_start(sel16_sb[:], sel16_d.ap())
            selb_sb = consts.tile([16, D], BF16, tag="selb")
            nc.sync.dma_start(selb_sb[:], selb_d.ap())
            bo128 = consts.tile([128, D], BF16, tag="bo128")
            nc.sync.dma_start(bo128[:], bo128_d.ap())
            wo_sb = wpool.tile([128, KT, D], BF16, tag="wo")

            kT_sb = consts.tile([128, CT, SKV], BF16, tag="kT")
            v_sb = consts.tile([SKV, H, DH], BF16, tag="v")

            # ---- k projection: kT[c, s] = sum_k wk[k, c] yT[k, s]; fold (.+bk)/8 ----
            for ct in range(CT):
                psk = pmm.tile([128, CH], F32, tag="mm")
                for kt in range(KC):
                    nc.tensor.matmul(
                        psk[:, 0:SKV],
                        wk_sb[:, kt, ct * 128:(ct + 1) * 128],
                        yt_sb[:, kt, :],
                        start=(kt == 0),
                        stop=(kt == KC - 1),
                    )
                nc.scalar.activation(
                    kT_sb[:, ct, :],
                    psk[:, 0:SKV],
                    AF.Identity,
                    scale=0.125,
                    bias=bk8_sb[:, ct:ct + 1],
                )

            # ---- v projection: v[s, c] = sum_k yT[k, s] wv[k, c] + bv[c] ----
            for n in range(2):
                psv = pmm.tile([128, CH], F32, tag="mm")
                for kt in range(KC):
                    nc.tensor.matmul(
                        psv[0:SKV, :],
                        yt_sb[:, kt, :],
                        wv_sb[:, kt, n * 512:(n + 1) * 512],
                        start=(kt == 0),
                        stop=False,
                    )
                nc.tensor.matmul(
                    psv[0:SKV, :],
                    ones77r[:],
                    bv_sb[0:1, n * 512:(n + 1) * 512],
                    start=False,
                    stop=True,
                )
                nc.vector.tensor_copy(
                    v_sb[:, n * 8:(n + 1) * 8, :], psv[0:SKV, :]
                )

            # ---- software-pipelined main loop ----
            # iteration c: Q-proj(c) + attention(c), with O-proj(c-1) tiles
            # interleaved into the PE stream to keep it dense while ACT/DVE
            # work on softmax.
            aoT_tiles = [None, None]
            xT_tiles = {}

            def fetch_x(cc):
                if cc >= NCH or cc in xT_tiles:
                    return
                t = xpool.tile([128, KT, CH], BF16, tag="xT")
                nc.sync.dma_start(
                    t[:],
                    xt.ap().rearrange("(kt p) q -> p kt q", p=128)[
                        :, :, cc * CH:(cc + 1) * CH],
                )
                xT_tiles[cc] = t

            # first two query chunks before the (late-needed) wo load
            fetch_x(0)
            fetch_x(1)
            nc.sync.dma_start(wo_sb[:], wo_d.ap().rearrange("(kt p) n -> p kt n", p=128))

            def emit_op_tile(cc, t):
                """O-proj tile t=(qt*2+n) of chunk cc: 8 matmuls + DVE bias-add + DMA."""
                qt, n = t // 2, t % 2
                aoT_p = aoT_tiles[cc % 2]
                q0 = cc * CH
                pso = pmm.tile([128, CH], F32, tag="mm")
                for kt in range(KT):
                    nc.tensor.matmul(
                        pso[:],
                        aoT_p[:, kt, qt * 128:(qt + 1) * 128],
                        wo_sb[:, kt, n * 512:(n + 1) * 512],
                        start=(kt == 0),
                        stop=(kt == KT - 1),
                        skip_group_check=True,
                    )
                o_sb = opool.tile([128, CH], F32, tag="o")
                nc.vector.tensor_add(o_sb[:], pso[:], bo128[:, n * 512:(n + 1) * 512])
                nc.sync.dma_start(
                    out_d.ap()[q0 + qt * 128: q0 + (qt + 1) * 128,
                               n * 512:(n + 1) * 512],
                    o_sb[:],
                )

            for c in range(NCH + 1):
                op_t = [0]  # next O-proj tile index for chunk c-1

                def drain_op(upto, _c=c, _op_t=op_t):
                    if _c == 0:
                        return
                    while _op_t[0] < upto:
                        emit_op_tile(_c - 1, _op_t[0])
                        _op_t[0] += 1

                if c == NCH:
                    drain_op(8)
                    break

                if c == 0:
                    fetch_x(0)
                fetch_x(c + 1)
                xT_ch = xT_tiles.pop(c)

                # q^T projection (per column-tile); bias via per-partition
                # activation bias on the PSUM->SBUF copy (no PE matmul).
                qT = qpool.tile([128, CT, CH], BF16, tag="qT")
                for ct in range(CT):
                    psq = pmm.tile([128, CH], F32, tag="mm")
                    for kt in range(KT):
                        nc.tensor.matmul(
                            psq[:],
                            wq_sb[:, kt, ct * 128:(ct + 1) * 128],
                            xT_ch[:, kt, :],
                            start=(kt == 0),
                            stop=(kt == KT - 1),
                            skip_group_check=True,
                        )
                    nc.scalar.activation(
                        qT[:, ct, :],
                        psq[:],
                        AF.Identity,
                        bias=bqc_sb[:, ct:ct + 1],
                    )

                # attention pass A: scores -> exp -> sum-collect [16, CH]
                e_ch = epool.tile([SKV, H, CH], BF16, tag="e")
                ps_sum = pnrm.tile([16, CH], F32, tag="nrm")
                for h in range(H):
                    pssc = psc.tile([SKV, CH], F32, tag="sc")
                    nc.tensor.matmul(
                        pssc[:],
                        kT_sb[(h % 2) * 64:(h % 2) * 64 + 64, h // 2, :],
                        qT[(h % 2) * 64:(h % 2) * 64 + 64, h // 2, :],
                        start=True, stop=True, skip_group_check=True,
                    )
                    nc.scalar.activation(e_ch[:, h, :], pssc[:], AF.Exp)
                    nc.tensor.matmul(
                        ps_sum[:], sel16_sb[:, h * 16:(h + 1) * 16], e_ch[:, h, :],
                        start=(h == 0), stop=(h == H - 1), skip_group_check=True,
                    )
                    if h in (3, 7, 11):
                        drain_op({3: 1, 7: 2, 11: 3}[h])

                r16f = rp.tile([16, CH], F32, tag="rf")
                nc.vector.reciprocal_approx_fast(r16f[:], ps_sum[:])
                r16 = rp.tile([16, CH], BF16, tag="r")
                with nc.allow_low_precision(reason="softmax recip in bf16"):
                    nc.vector.tensor_copy(r16[:], r16f[:])
                # 4 O-proj tiles bridge the reciprocal's DVE latency so the
                # PE never idles long enough to trip the HAM re-throttle.
                drain_op(7)

                # pass B: rb = selb^T @ r (broadcast 1/sum to 128 rows), PV,
                # normalize into aoT.
                aoT = aopool.tile([128, KT, CH], BF16, tag="aoT")
                aoT_tiles[c % 2] = aoT
                for hp in range(H // 2):
                    rb_ps = pnrm.tile([128, CH], F32, tag="nrm")
                    nc.tensor.matmul(
                        rb_ps[:],
                        selb_sb[:, hp * 128:(hp + 1) * 128],
                        r16[:],
                        start=True, stop=True, skip_group_check=True,
                    )
                    rb_sb = rbpool.tile([128, CH], F32, tag="rb")
                    nc.scalar.activation(rb_sb[:], rb_ps[:], AF.Identity)
                    pspv = ppv.tile([128, CH], F32, tag="pv")
                    for half in range(2):
                        h = 2 * hp + half
                        nc.tensor.matmul(
                            pspv[half * 64:(half + 1) * 64, :],
                            v_sb[:, h, :],
                            e_ch[:, h, :],
                            start=True, stop=True, skip_group_check=True,
                        )
                    nc.vector.tensor_mul(aoT[:, hp, :], pspv[:], rb_sb[:])
                    if hp == 3:
                        drain_op(8)

    nc.compile()
    return nc


def _get_nc():
    if "nc" not in _CACHED:
        _CACHED["nc"] = _build()
    return _CACHED["nc"]


def _prep_inmaps(x, y, wq, bq, wk, bk, wv, bv, wo, bo):
    x = np.asarray(x)
    y = np.asarray(y)
    bf = ml_dtypes.bfloat16
    wq_b = np.asarray(wq).astype(bf)
    wk_b = np.asarray(wk).astype(bf)
    wv_b = np.asarray(wv).astype(bf)
    wo_b = np.asarray(wo).astype(bf)
    bv_b = np.asarray(bv).reshape(1, D).astype(bf)
    bo128 = np.ascontiguousarray(
        np.broadcast_to(np.asarray(bo).reshape(1, D), (128, D))).astype(bf)
    bqc = np.asarray(bq).reshape(1, D).astype(np.float32)
    bk8 = (np.asarray(bk).reshape(1, D) * 0.125).astype(np.float32)
    sel16 = np.zeros((SKV, H, 16), np.float32)
    sel16[:, np.arange(H), np.arange(16)] = 1.0
    sel16 = sel16.reshape(SKV, H * 16).astype(bf)
    # selb[i, hp*128 + p] = 1 iff i == 2*hp + (p >= 64)
    selb = np.zeros((16, H // 2, 128), np.float32)
    for hp in range(H // 2):
        selb[2 * hp, hp, 0:64] = 1.0
        selb[2 * hp + 1, hp, 64:128] = 1.0
    selb = selb.reshape(16, D).astype(bf)

    in_maps = []
    for b in range(B):
        in_maps.append({
            "xt": np.ascontiguousarray(x[b].T).astype(bf),
            "yt": np.ascontiguousarray(y[b].T).astype(bf),
            "wq": wq_b, "wk": wk_b, "wv": wv_b, "wo": wo_b,
            "bqc": bqc, "bk8": bk8, "bv": bv_b, "bo128": bo128,
            "sel16": sel16, "selb": selb,
        })
    return in_maps


def kernel(x, y, wq, bq, wk, bk, wv, bv, wo, bo):
    in_maps = _prep_inmaps(x, y, wq, bq, wk, bk, wv, bv, wo, bo)
    nc = _get_nc()
    res = bass_utils.run_bass_kernel_spmd(nc, in_maps, core_ids=list(range(B)))
    out = np.stack([res.results[b]["out"] for b in range(B)], axis=0)
    return out.astype(np.float32)
